# revision 40
# baseline (speedup 1.0000x reference)
"""Trainium2 Bass kernel for ConvertedLlamaAttention (LoRA q/k/v + RoPE + causal attention + out-proj).

Strategy: tensor-parallel over heads across 8 NeuronCores (4 heads/core).
All device matmuls run in "transposed" layouts so no on-device transposes are
needed anywhere:
  - Q^T, K^T computed as W^T-stationary matmuls (head_dim on partitions),
  - V computed in natural layout (seq on partitions) from the same X^T tiles,
  - scores computed transposed (S^T = K^T^T-slices @ Q^T) so softmax sums are
    done with a ones-vector matmul, and A·V consumes V in natural layout,
  - out-proj consumes A·V^T directly as the stationary operand.
LoRA (incl. the half-interleave) is folded into the weights on the host.
Each core emits a partial (2048, 4096) output (row-parallel Wo); the host sums.

v2 perf notes:
  - phase-1 matmuls issued in same-PSUM-bank runs of 16 (m outer, k inner)
    to avoid per-matmul PSUM bank cycling,
  - q/k/v, probs, avt, Wo and the output all in bf16 (2x DVE/ACT, half DMA),
  - causal structure exploited at 128 granularity in the A.V and row-sum
    matmuls (rhs narrowed on diagonal blocks); masks only on 128x128 corners,
  - Wo preloaded into SBUF up front; PSUM->SBUF out-proj copies spread over
    scalar/vector/gpsimd engines.
"""
import sys

for _p in ("/opt/trn_rl_repo", "/root/.axon_site/_ro/trn_rl_repo"):
    if _p not in sys.path:
        sys.path.insert(0, _p)

import numpy as np
import ml_dtypes

import concourse.bass as bass  # noqa: F401  (registers types)
import concourse.mybir as mybir
import concourse.tile as tile
from concourse import bacc, bass_utils

F32 = mybir.dt.float32
BF16 = mybir.dt.bfloat16

H = 4096          # hidden
S = 2048          # sequence
P = 128           # partitions
HD = 128          # head dim
NCORES = 8
HPC = 4           # heads per core
CW = HPC * HD     # per-core width of q/k/v/attn dims = 512
NCHUNKS = 4       # seq chunks of 512
KCH = H // P      # 32 hidden chunks
LORA_SCALING = 2.0
EXP_SCALE = float(1.0 / np.sqrt(HD))

_CACHE = {}


def _build():
    nc = bacc.Bacc("TRN2", target_bir_lowering=False, debug=False, num_devices=NCORES)

    xt_d = nc.declare_dram_parameter("xt", [H, S], BF16, isOutput=False)
    wq_d = nc.declare_dram_parameter("wq", [H, CW], BF16, isOutput=False)
    wk_d = nc.declare_dram_parameter("wk", [H, CW], BF16, isOutput=False)
    wv_d = nc.declare_dram_parameter("wv", [H, CW], BF16, isOutput=False)
    wot_d = nc.declare_dram_parameter("wot", [CW, H], BF16, isOutput=False)
    cs_d = nc.declare_dram_parameter("cs", [P, S], F32, isOutput=False)
    tri_d = nc.declare_dram_parameter("tri", [P, P], BF16, isOutput=False)
    neye_d = nc.declare_dram_parameter("neye", [P, P], BF16, isOutput=False)
    ones_d = nc.declare_dram_parameter("ones", [P, P], BF16, isOutput=False)
    out_d = nc.declare_dram_parameter("out", [S, H], BF16, isOutput=True)

    xt3 = xt_d.rearrange("(ko p) s -> p ko s", p=P)      # (128, 32, 2048)
    wq3 = wq_d.rearrange("(ko p) m -> p ko m", p=P)      # (128, 32, 512)
    wk3 = wk_d.rearrange("(ko p) m -> p ko m", p=P)
    wv3 = wv_d.rearrange("(ko p) m -> p ko m", p=P)
    wot3 = wot_d.rearrange("(h p) n -> p h n", p=P)      # (128, 4, 4096)

    with tile.TileContext(nc) as tc:
        with tc.tile_pool(name="persist", bufs=1) as pp:
            qt = [pp.tile([P, S], BF16, tag=f"qt{h}", name=f"qt{h}") for h in range(HPC)]
            kt = [pp.tile([P, S], BF16, tag=f"kt{h}", name=f"kt{h}") for h in range(HPC)]
            v_sb = pp.tile([P, S // P, CW], BF16, tag="v")   # (128, 16, 512)
            cs_sb = pp.tile([P, S], F32, tag="cs")
            ones_sb = pp.tile([P, P], BF16, tag="ones")
            tri_sb = pp.tile([P, P], BF16, tag="tri")
            neye_sb = pp.tile([P, P], BF16, tag="neye")
            wot_sb = pp.tile([P, HPC, H], BF16, tag="wot")   # (128, 4, 4096)
            nc.scalar.dma_start(cs_sb[:], cs_d[:])
            nc.scalar.dma_start(ones_sb[:], ones_d[:])
            nc.scalar.dma_start(tri_sb[:], tri_d[:])
            nc.scalar.dma_start(neye_sb[:], neye_d[:])
            ones_col = ones_sb[:, 0:1]
            ones_row = ones_sb[0:1, :]

            def rope(qp, dest, ncx):
                sl = slice(ncx * 512, (ncx + 1) * 512)
                t1 = ropep.tile([P, 512], F32, tag="r1")
                t2 = ropep.tile([P, 512], F32, tag="r2")
                # dest[0:64]  = q1*cos - q2*sin ; dest[64:] = q1*sin + q2*cos
                nc.vector.tensor_mul(t1[0:64], qp[0:64], cs_sb[0:64, sl])
                nc.vector.tensor_mul(t2[0:64], qp[64:128], cs_sb[64:128, sl])
                nc.vector.tensor_sub(dest[0:64], t1[0:64], t2[0:64])
                nc.vector.tensor_mul(t1[64:128], qp[0:64], cs_sb[64:128, sl])
                nc.vector.tensor_mul(t2[64:128], qp[64:128], cs_sb[0:64, sl])
                nc.vector.tensor_add(dest[64:128], t1[64:128], t2[64:128])

            # ---------------- Phase 1: Q^T, K^T, V projections ----------------
            # Matmuls run in same-PSUM-bank runs of 16 (m outer, hidden-chunk
            # inner) so the PE never cycles output banks between instructions.
            with tc.tile_pool(name="xtp", bufs=15) as xtp, \
                 tc.tile_pool(name="wp", bufs=12) as wp, \
                 tc.tile_pool(name="ropep", bufs=2) as ropep, \
                 tc.tile_pool(name="projps", bufs=8, space="PSUM") as projps:
                wdma = (nc.sync, nc.scalar)
                for ncx in range(NCHUNKS):
                    ssl = slice(ncx * 512, (ncx + 1) * 512)
                    xts = []
                    for b in range(8):
                        t = xtp.tile([P, 4, 512], BF16, tag="xt")
                        nc.gpsimd.dma_start(t[:], xt3[:, b * 4:(b + 1) * 4, ssl])
                        xts.append(t)
                    # spread the Wo preload across phase 1 (behind this
                    # chunk's xt tiles on the gpsimd queue) so it doesn't
                    # compete with the startup-critical x/w loads
                    nc.gpsimd.dma_start(
                        wot_sb[:, :, ncx * 1024:(ncx + 1) * 1024],
                        wot3[:, :, ncx * 1024:(ncx + 1) * 1024])

                    def wtiles(w3):
                        # 512KB pieces on alternating queues: the first
                        # matmuls only wait on the first piece
                        ws = []
                        for j in range(8):
                            w_t = wp.tile([P, 4, CW], BF16, tag="w")
                            wdma[j % 2].dma_start(w_t[:], w3[:, 4 * j:4 * j + 4, :])
                            ws.append(w_t)
                        return ws

                    for wsel, w3 in (("q", wq3), ("k", wk3)):
                        ps = [projps.tile([P, 512], F32, tag="proj",
                                          name=f"{wsel}_ps{ncx}_{i}") for i in range(HPC)]
                        ws = wtiles(w3)
                        for half in range(2):
                            for m in range(HPC):
                                for ki in range(16):
                                    k = 16 * half + ki
                                    rhs = xts[k // 4][:, k % 4, :]
                                    nc.tensor.matmul(
                                        ps[m][:],
                                        lhsT=ws[k // 4][:, k % 4, m * HD:(m + 1) * HD],
                                        rhs=rhs, start=(k == 0), stop=(k == KCH - 1))
                        dst = qt if wsel == "q" else kt
                        for m in range(HPC):
                            rope(ps[m], dst[m][:, ssl], ncx)

                    v_ps = [projps.tile([P, 512], F32, tag="proj",
                                        name=f"v_ps{ncx}_{i}") for i in range(4)]
                    ws = wtiles(wv3)
                    for half in range(2):
                        for t in range(4):
                            for ki in range(16):
                                k = 16 * half + ki
                                nc.tensor.matmul(
                                    v_ps[t][:],
                                    lhsT=xts[k // 4][:, k % 4, t * P:(t + 1) * P],
                                    rhs=ws[k // 4][:, k % 4, :],
                                    start=(k == 0), stop=(k == KCH - 1))
                    for t in range(4):
                        nc.scalar.copy(v_sb[:, ncx * 4 + t, :], v_ps[t][:])

            # ---------------- Phase 2: attention ----------------
            # Scores/probs processed in double-width (2 k-tiles) units so each
            # Exp activation covers 1024 elements/partition (halves ACT
            # overhead).  A.V and row-sum matmuls are narrowed on diagonal
            # blocks (queries < key-tile start contribute nothing); only the
            # 128x128 corner of each diagonal tile needs masking.
            with tc.tile_pool(name="avtsp", bufs=18) as avtsp:
                avt_all = [[None] * HPC for _ in range(NCHUNKS)]

                with tc.tile_pool(name="probsp", bufs=8) as probsp, \
                     tc.tile_pool(name="recp", bufs=2) as recp, \
                     tc.tile_pool(name="stps", bufs=2, space="PSUM") as stps, \
                     tc.tile_pool(name="avtps", bufs=2, space="PSUM") as avtps, \
                     tc.tile_pool(name="smallps", bufs=2, space="PSUM") as smallps:

                    def emit_scores(qc, ktb, hs):
                        qbase = qc * 512
                        out = []
                        for h in hs:
                            st2 = stps.tile([P, 2, 512], F32, tag="st")
                            for u in range(2):
                                kti = 2 * ktb + u
                                r = kti - 4 * qc
                                q_off = 128 * r if r >= 1 else 0
                                nc.tensor.matmul(
                                    st2[:, u, q_off:512],
                                    lhsT=kt[h][:, kti * P:(kti + 1) * P],
                                    rhs=qt[h][:, qbase + q_off:qbase + 512],
                                    start=True, stop=(r < 0))
                                if r >= 0:
                                    # add -2000 onto the upper-triangle corner
                                    # so Exp underflows to 0 there
                                    csl = slice(r * 128, (r + 1) * 128)
                                    nc.tensor.matmul(
                                        st2[:, u, csl], lhsT=neye_sb[:],
                                        rhs=tri_sb[:], start=False,
                                        stop=True, skip_group_check=True)
                            # Exp reads the full pair; columns the narrowed
                            # scores never wrote are garbage but unconsumed
                            probs2 = probsp.tile([P, 2, 512], BF16, tag="probs")
                            nc.scalar.activation(probs2[:], st2[:],
                                                 mybir.ActivationFunctionType.Exp,
                                                 scale=EXP_SCALE)
                            out.append((h, probs2))
                        return out

                    def emit_av_half(h, tiles, avt_ps, sums_ps, qc, nkt):
                        # same-type matmuls back to back: all A.V for a head
                        # into one PSUM bank, then all sums
                        for mm in ("av", "sums"):
                            for ktb, probs2 in tiles:
                                for u in range(2):
                                    kti = 2 * ktb + u
                                    r = kti - 4 * qc
                                    q_off = 128 * r if r >= 1 else 0
                                    psl = slice(q_off, 512)
                                    dst = avt_ps[h] if mm == "av" else sums_ps[h]
                                    lhsT = (v_sb[:, kti, h * HD:(h + 1) * HD]
                                            if mm == "av" else ones_sb[:])
                                    nc.tensor.matmul(
                                        dst[:, psl], lhsT=lhsT,
                                        rhs=probs2[:, u, psl],
                                        start=(kti == 0), stop=(kti == nkt - 1))

                    def emit_epilogue(hs, avt_ps, sums_ps, qc):
                        # reciprocal of the (replicated) sums, then normalize
                        # straight out of PSUM - 4 vector ops per head
                        for h in hs:
                            recip_f = recp.tile([P, 512], F32, tag="recf",
                                                name=f"recf{qc}_{h}")
                            scratch = recp.tile([P, 512], F32, tag="recs",
                                                name=f"recs{qc}_{h}")
                            nc.vector.reciprocal_approx_accurate(
                                out=recip_f[:], in_=sums_ps[h][:],
                                scratch=scratch[:])
                            avs = avtsp.tile([P, 512], BF16, tag="avts",
                                             name=f"avts{qc}_{h}")
                            nc.vector.tensor_mul(avs[:], avt_ps[h][:], recip_f[:])
                            avt_all[qc][h] = avs

                    # flat cross-group software pipeline: each group's tail
                    # A.V half + epilogue are deferred into the next group,
                    # emitted after its first scores pair so the stps/Exp
                    # refill latency is hidden
                    tail = []
                    for qc in range(NCHUNKS):
                        nkt = 4 * (qc + 1)
                        for hp in range(HPC // 2):
                            hs = (2 * hp, 2 * hp + 1)
                            avt_ps = {h: avtps.tile([P, 512], F32, tag="avt",
                                                    name=f"avt{qc}_{h}")
                                      for h in hs}
                            sums_ps = {h: smallps.tile([P, 512], F32, tag="small",
                                                       name=f"sums{qc}_{h}")
                                       for h in hs}
                            pend = {h: [] for h in hs}
                            for idx in range(nkt // 2):
                                cur = emit_scores(qc, idx, hs)
                                if idx < 2 and tail:
                                    tail.pop(0)()
                                hsel = hs[idx % 2]
                                if pend[hsel]:
                                    emit_av_half(hsel, pend[hsel], avt_ps,
                                                 sums_ps, qc, nkt)
                                    pend[hsel] = []
                                for h, probs2 in cur:
                                    pend[h].append((idx, probs2))

                            def make_tail(hs=hs, pend=pend, avt_ps=avt_ps,
                                          sums_ps=sums_ps, qc=qc, nkt=nkt):
                                # two parts so the next group's first two
                                # scores slots each get fill work
                                def cl_a():
                                    for h in hs:
                                        if pend[h]:
                                            emit_av_half(h, pend[h], avt_ps,
                                                         sums_ps, qc, nkt)
                                def cl_b():
                                    emit_epilogue(hs, avt_ps, sums_ps, qc)
                                return [cl_a, cl_b]
                            tail = make_tail()
                    for cl in tail:
                        cl()

                # ---------------- Phase 3: out-proj ----------------
                with tc.tile_pool(name="osbp", bufs=4) as osbp, \
                     tc.tile_pool(name="outps", bufs=8, space="PSUM") as outps:
                    dma_engines = (nc.scalar, nc.sync)
                    cp_i = 0
                    for hc in range(8):
                        for qc in range(NCHUNKS):
                            for qs in range(4):
                                o_ps = outps.tile([P, 512], F32, tag="o")
                                for h in range(HPC):
                                    nc.tensor.matmul(
                                        o_ps[:],
                                        lhsT=avt_all[qc][h][:, qs * P:(qs + 1) * P],
                                        rhs=wot_sb[:, h, hc * 512:(hc + 1) * 512],
                                        start=(h == 0), stop=(h == HPC - 1))
                                o_sb = osbp.tile([P, 512], BF16, tag="osb")
                                nc.vector.tensor_copy(o_sb[:], o_ps[:])
                                dma_engines[cp_i % 2].dma_start(
                                    out_d[qc * 512 + qs * P: qc * 512 + (qs + 1) * P,
                                          hc * 512:(hc + 1) * 512],
                                    o_sb[:])
                                cp_i += 1

    nc.compile()
    return nc


def _fold(W, A, B):
    """Fold LoRA + its half/interleave permutation into the base weight."""
    BA = (B.astype(np.float64) @ A.astype(np.float64)) * LORA_SCALING
    j = np.arange(H)
    g = np.where(j < H // 2, 2 * j, 2 * (j - H // 2) + 1)
    return (W.astype(np.float64) + BA[g, :]).astype(np.float32)


def _host_consts():
    inv_freq = (1.0 / (10000.0 ** (np.arange(0, HD, 2, dtype=np.float32) / HD))).astype(np.float32)
    freqs = np.arange(S, dtype=np.float32)[:, None] * inv_freq[None, :]   # (S, 64)
    cs = np.concatenate([np.cos(freqs).T, np.sin(freqs).T], axis=0).astype(np.float32)  # (128, S)
    p = np.arange(P)[:, None]
    f = np.arange(P)[None, :]
    tri = (p > f).astype(ml_dtypes.bfloat16)              # (128, 128) kill-mask
    neye = (-2000.0 * np.eye(P)).astype(ml_dtypes.bfloat16)
    ones = np.ones((P, P), dtype=ml_dtypes.bfloat16)
    return cs, tri, neye, ones


def kernel(hidden_states, Wq, Wk, Wv, Wo, Aq, Bq, Ak, Bk, Av, Bv):
    if "nc" not in _CACHE:
        _CACHE["nc"] = _build()
    nc = _CACHE["nc"]

    x = np.ascontiguousarray(np.asarray(hidden_states, dtype=np.float32)[0])  # (S, H)
    xt_bf = np.ascontiguousarray(x.T).astype(ml_dtypes.bfloat16)

    Wq_eff = _fold(np.asarray(Wq), np.asarray(Aq), np.asarray(Bq))
    Wk_eff = _fold(np.asarray(Wk), np.asarray(Ak), np.asarray(Bk))
    Wv_eff = _fold(np.asarray(Wv), np.asarray(Av), np.asarray(Bv))
    Wo_np = np.asarray(Wo, dtype=np.float32)

    cs, tri, neye, ones = _host_consts()

    in_maps = []
    for c in range(NCORES):
        cols = slice(CW * c, CW * (c + 1))
        in_maps.append({
            "xt": xt_bf,
            "wq": np.ascontiguousarray(Wq_eff[cols].T).astype(ml_dtypes.bfloat16),
            "wk": np.ascontiguousarray(Wk_eff[cols].T).astype(ml_dtypes.bfloat16),
            "wv": np.ascontiguousarray(Wv_eff[cols].T).astype(ml_dtypes.bfloat16),
            "wot": np.ascontiguousarray(Wo_np[:, cols].T).astype(ml_dtypes.bfloat16),
            "cs": cs,
            "tri": tri,
            "neye": neye,
            "ones": ones,
        })
    _CACHE["in_maps"] = in_maps

    res = bass_utils.run_bass_kernel_spmd(nc, in_maps, core_ids=list(range(NCORES)))
    acc = np.zeros((S, H), dtype=np.float32)
    for c in range(NCORES):
        acc += res.results[c]["out"].astype(np.float32)
    return acc[None]


# revision 44
# speedup vs baseline: 1.0032x; 1.0032x over previous
"""Trainium2 Bass kernel for ConvertedLlamaAttention (LoRA q/k/v + RoPE + causal attention + out-proj).

Strategy: tensor-parallel over heads across 8 NeuronCores (4 heads/core).
All device matmuls run in "transposed" layouts so no on-device transposes are
needed anywhere:
  - Q^T, K^T computed as W^T-stationary matmuls (head_dim on partitions),
  - V computed in natural layout (seq on partitions) from the same X^T tiles,
  - scores computed transposed (S^T = K^T^T-slices @ Q^T) so softmax sums are
    done with a ones-vector matmul, and A·V consumes V in natural layout,
  - out-proj consumes A·V^T directly as the stationary operand.
LoRA (incl. the half-interleave) is folded into the weights on the host.
Each core emits a partial (2048, 4096) output (row-parallel Wo); the host sums.

v2 perf notes:
  - phase-1 matmuls issued in same-PSUM-bank runs of 16 (m outer, k inner)
    to avoid per-matmul PSUM bank cycling,
  - q/k/v, probs, avt, Wo and the output all in bf16 (2x DVE/ACT, half DMA),
  - causal structure exploited at 128 granularity in the A.V and row-sum
    matmuls (rhs narrowed on diagonal blocks); masks only on 128x128 corners,
  - Wo preloaded into SBUF up front; PSUM->SBUF out-proj copies spread over
    scalar/vector/gpsimd engines.
"""
import sys

for _p in ("/opt/trn_rl_repo", "/root/.axon_site/_ro/trn_rl_repo"):
    if _p not in sys.path:
        sys.path.insert(0, _p)

import numpy as np
import ml_dtypes

import concourse.bass as bass  # noqa: F401  (registers types)
import concourse.mybir as mybir
import concourse.tile as tile
from concourse import bacc, bass_utils

F32 = mybir.dt.float32
BF16 = mybir.dt.bfloat16

H = 4096          # hidden
S = 2048          # sequence
P = 128           # partitions
HD = 128          # head dim
NCORES = 8
HPC = 4           # heads per core
CW = HPC * HD     # per-core width of q/k/v/attn dims = 512
NCHUNKS = 4       # seq chunks of 512
KCH = H // P      # 32 hidden chunks
LORA_SCALING = 2.0
EXP_SCALE = float(1.0 / np.sqrt(HD))

_CACHE = {}


def _build():
    nc = bacc.Bacc("TRN2", target_bir_lowering=False, debug=False, num_devices=NCORES)

    xt_d = nc.declare_dram_parameter("xt", [H, S], BF16, isOutput=False)
    wq_d = nc.declare_dram_parameter("wq", [H, CW], BF16, isOutput=False)
    wk_d = nc.declare_dram_parameter("wk", [H, CW], BF16, isOutput=False)
    wv_d = nc.declare_dram_parameter("wv", [H, CW], BF16, isOutput=False)
    wot_d = nc.declare_dram_parameter("wot", [CW, H], BF16, isOutput=False)
    cs_d = nc.declare_dram_parameter("cs", [P, S], F32, isOutput=False)
    tri_d = nc.declare_dram_parameter("tri", [P, P], BF16, isOutput=False)
    neye_d = nc.declare_dram_parameter("neye", [P, P], BF16, isOutput=False)
    ones_d = nc.declare_dram_parameter("ones", [P, P], BF16, isOutput=False)
    out_d = nc.declare_dram_parameter("out", [S, H], BF16, isOutput=True)

    xt3 = xt_d.rearrange("(ko p) s -> p ko s", p=P)      # (128, 32, 2048)
    wq3 = wq_d.rearrange("(ko p) m -> p ko m", p=P)      # (128, 32, 512)
    wk3 = wk_d.rearrange("(ko p) m -> p ko m", p=P)
    wv3 = wv_d.rearrange("(ko p) m -> p ko m", p=P)
    wot3 = wot_d.rearrange("(h p) n -> p h n", p=P)      # (128, 4, 4096)

    with tile.TileContext(nc) as tc:
        with tc.tile_pool(name="persist", bufs=1) as pp:
            qt = [pp.tile([P, S], BF16, tag=f"qt{h}", name=f"qt{h}") for h in range(HPC)]
            kt = [pp.tile([P, S], BF16, tag=f"kt{h}", name=f"kt{h}") for h in range(HPC)]
            v_sb = pp.tile([P, S // P, CW], BF16, tag="v")   # (128, 16, 512)
            cs_sb = pp.tile([P, S], F32, tag="cs")
            ones_sb = pp.tile([P, P], BF16, tag="ones")
            tri_sb = pp.tile([P, P], BF16, tag="tri")
            neye_sb = pp.tile([P, P], BF16, tag="neye")
            wot_sb = pp.tile([P, HPC, H], BF16, tag="wot")   # (128, 4, 4096)
            def load_consts():
                # deferred to chunk 1 so these don't sit ahead of the
                # startup-critical weight pieces on the scalar DMA queue
                # (cs is first read by rope at ~75us, tri/neye in phase 2)
                nc.scalar.dma_start(cs_sb[:], cs_d[:])
                nc.scalar.dma_start(ones_sb[:], ones_d[:])
                nc.scalar.dma_start(tri_sb[:], tri_d[:])
                nc.scalar.dma_start(neye_sb[:], neye_d[:])

            def rope(qp, dest, ncx):
                sl = slice(ncx * 512, (ncx + 1) * 512)
                t1 = ropep.tile([P, 512], F32, tag="r1")
                t2 = ropep.tile([P, 512], F32, tag="r2")
                # dest[0:64]  = q1*cos - q2*sin ; dest[64:] = q1*sin + q2*cos
                nc.vector.tensor_mul(t1[0:64], qp[0:64], cs_sb[0:64, sl])
                nc.vector.tensor_mul(t2[0:64], qp[64:128], cs_sb[64:128, sl])
                nc.vector.tensor_sub(dest[0:64], t1[0:64], t2[0:64])
                nc.vector.tensor_mul(t1[64:128], qp[0:64], cs_sb[64:128, sl])
                nc.vector.tensor_mul(t2[64:128], qp[64:128], cs_sb[0:64, sl])
                nc.vector.tensor_add(dest[64:128], t1[64:128], t2[64:128])

            # ---------------- Phase 1: Q^T, K^T, V projections ----------------
            # Matmuls run in same-PSUM-bank runs of 16 (m outer, hidden-chunk
            # inner) so the PE never cycles output banks between instructions.
            with tc.tile_pool(name="xtp", bufs=15) as xtp, \
                 tc.tile_pool(name="wp", bufs=12) as wp, \
                 tc.tile_pool(name="ropep", bufs=2) as ropep, \
                 tc.tile_pool(name="projps", bufs=8, space="PSUM") as projps:
                wdma = (nc.sync, nc.scalar)
                for ncx in range(NCHUNKS):
                    ssl = slice(ncx * 512, (ncx + 1) * 512)
                    xts = []
                    for b in range(8):
                        t = xtp.tile([P, 4, 512], BF16, tag="xt")
                        nc.gpsimd.dma_start(t[:], xt3[:, b * 4:(b + 1) * 4, ssl])
                        xts.append(t)
                    # spread the Wo preload across phase 1 (behind this
                    # chunk's xt tiles on the gpsimd queue) so it doesn't
                    # compete with the startup-critical x/w loads
                    nc.gpsimd.dma_start(
                        wot_sb[:, :, ncx * 1024:(ncx + 1) * 1024],
                        wot3[:, :, ncx * 1024:(ncx + 1) * 1024])

                    def wtiles(w3):
                        # 512KB pieces on alternating queues: the first
                        # matmuls only wait on the first piece
                        ws = []
                        for j in range(8):
                            w_t = wp.tile([P, 4, CW], BF16, tag="w")
                            wdma[j % 2].dma_start(w_t[:], w3[:, 4 * j:4 * j + 4, :])
                            ws.append(w_t)
                        return ws

                    for wsel, w3 in (("q", wq3), ("k", wk3)):
                        ps = [projps.tile([P, 512], F32, tag="proj",
                                          name=f"{wsel}_ps{ncx}_{i}") for i in range(HPC)]
                        ws = wtiles(w3)
                        if ncx == 0 and wsel == "q":
                            load_consts()
                        for half in range(2):
                            for m in range(HPC):
                                for ki in range(16):
                                    k = 16 * half + ki
                                    rhs = xts[k // 4][:, k % 4, :]
                                    nc.tensor.matmul(
                                        ps[m][:],
                                        lhsT=ws[k // 4][:, k % 4, m * HD:(m + 1) * HD],
                                        rhs=rhs, start=(k == 0), stop=(k == KCH - 1))
                        dst = qt if wsel == "q" else kt
                        for m in range(HPC):
                            rope(ps[m], dst[m][:, ssl], ncx)

                    v_ps = [projps.tile([P, 512], F32, tag="proj",
                                        name=f"v_ps{ncx}_{i}") for i in range(4)]
                    ws = wtiles(wv3)
                    for half in range(2):
                        for t in range(4):
                            for ki in range(16):
                                k = 16 * half + ki
                                nc.tensor.matmul(
                                    v_ps[t][:],
                                    lhsT=xts[k // 4][:, k % 4, t * P:(t + 1) * P],
                                    rhs=ws[k // 4][:, k % 4, :],
                                    start=(k == 0), stop=(k == KCH - 1))
                    for t in range(4):
                        nc.scalar.copy(v_sb[:, ncx * 4 + t, :], v_ps[t][:])

            # ---------------- Phase 2: attention ----------------
            # Scores/probs processed in double-width (2 k-tiles) units so each
            # Exp activation covers 1024 elements/partition (halves ACT
            # overhead).  A.V and row-sum matmuls are narrowed on diagonal
            # blocks (queries < key-tile start contribute nothing); only the
            # 128x128 corner of each diagonal tile needs masking.
            with tc.tile_pool(name="avtsp", bufs=18) as avtsp:
                avt_all = [[None] * HPC for _ in range(NCHUNKS)]

                with tc.tile_pool(name="probsp", bufs=8) as probsp, \
                     tc.tile_pool(name="recp", bufs=2) as recp, \
                     tc.tile_pool(name="stps", bufs=2, space="PSUM") as stps, \
                     tc.tile_pool(name="avtps", bufs=2, space="PSUM") as avtps, \
                     tc.tile_pool(name="smallps", bufs=2, space="PSUM") as smallps:

                    def emit_scores(qc, ktb, hs):
                        qbase = qc * 512
                        out = []
                        for h in hs:
                            st2 = stps.tile([P, 2, 512], F32, tag="st")
                            for u in range(2):
                                kti = 2 * ktb + u
                                r = kti - 4 * qc
                                q_off = 128 * r if r >= 1 else 0
                                nc.tensor.matmul(
                                    st2[:, u, q_off:512],
                                    lhsT=kt[h][:, kti * P:(kti + 1) * P],
                                    rhs=qt[h][:, qbase + q_off:qbase + 512],
                                    start=True, stop=(r < 0))
                                if r >= 0:
                                    # add -2000 onto the upper-triangle corner
                                    # so Exp underflows to 0 there
                                    csl = slice(r * 128, (r + 1) * 128)
                                    nc.tensor.matmul(
                                        st2[:, u, csl], lhsT=neye_sb[:],
                                        rhs=tri_sb[:], start=False,
                                        stop=True, skip_group_check=True)
                            # Exp reads the full pair; columns the narrowed
                            # scores never wrote are garbage but unconsumed
                            probs2 = probsp.tile([P, 2, 512], BF16, tag="probs")
                            nc.scalar.activation(probs2[:], st2[:],
                                                 mybir.ActivationFunctionType.Exp,
                                                 scale=EXP_SCALE)
                            out.append((h, probs2))
                        return out

                    def emit_av_half(h, tiles, avt_ps, sums_ps, qc, nkt):
                        # same-type matmuls back to back: all A.V for a head
                        # into one PSUM bank, then all sums
                        for mm in ("av", "sums"):
                            for ktb, probs2 in tiles:
                                for u in range(2):
                                    kti = 2 * ktb + u
                                    r = kti - 4 * qc
                                    q_off = 128 * r if r >= 1 else 0
                                    psl = slice(q_off, 512)
                                    dst = avt_ps[h] if mm == "av" else sums_ps[h]
                                    lhsT = (v_sb[:, kti, h * HD:(h + 1) * HD]
                                            if mm == "av" else ones_sb[:])
                                    nc.tensor.matmul(
                                        dst[:, psl], lhsT=lhsT,
                                        rhs=probs2[:, u, psl],
                                        start=(kti == 0), stop=(kti == nkt - 1))

                    def emit_epilogue(hs, avt_ps, sums_ps, qc):
                        # reciprocal of the (replicated) sums, then normalize
                        # straight out of PSUM - 4 vector ops per head
                        for h in hs:
                            recip_f = recp.tile([P, 512], F32, tag="recf",
                                                name=f"recf{qc}_{h}")
                            scratch = recp.tile([P, 512], F32, tag="recs",
                                                name=f"recs{qc}_{h}")
                            nc.vector.reciprocal_approx_accurate(
                                out=recip_f[:], in_=sums_ps[h][:],
                                scratch=scratch[:])
                            avs = avtsp.tile([P, 512], BF16, tag="avts",
                                             name=f"avts{qc}_{h}")
                            nc.vector.tensor_mul(avs[:], avt_ps[h][:], recip_f[:])
                            avt_all[qc][h] = avs

                    # flat cross-group software pipeline: each group's tail
                    # A.V half + epilogue are deferred into the next group,
                    # emitted after its first scores pair so the stps/Exp
                    # refill latency is hidden
                    tail = []
                    for qc in range(NCHUNKS):
                        nkt = 4 * (qc + 1)
                        for hp in range(HPC // 2):
                            hs = (2 * hp, 2 * hp + 1)
                            avt_ps = {h: avtps.tile([P, 512], F32, tag="avt",
                                                    name=f"avt{qc}_{h}")
                                      for h in hs}
                            sums_ps = {h: smallps.tile([P, 512], F32, tag="small",
                                                       name=f"sums{qc}_{h}")
                                       for h in hs}
                            pend = {h: [] for h in hs}
                            for idx in range(nkt // 2):
                                cur = emit_scores(qc, idx, hs)
                                if idx < 2 and tail:
                                    tail.pop(0)()
                                hsel = hs[idx % 2]
                                if pend[hsel]:
                                    emit_av_half(hsel, pend[hsel], avt_ps,
                                                 sums_ps, qc, nkt)
                                    pend[hsel] = []
                                for h, probs2 in cur:
                                    pend[h].append((idx, probs2))

                            def make_tail(hs=hs, pend=pend, avt_ps=avt_ps,
                                          sums_ps=sums_ps, qc=qc, nkt=nkt):
                                # two parts so the next group's first two
                                # scores slots each get fill work
                                def cl_a():
                                    for h in hs:
                                        if pend[h]:
                                            emit_av_half(h, pend[h], avt_ps,
                                                         sums_ps, qc, nkt)
                                def cl_b():
                                    emit_epilogue(hs, avt_ps, sums_ps, qc)
                                return [cl_a, cl_b]
                            tail = make_tail()
                    for cl in tail:
                        cl()

                # ---------------- Phase 3: out-proj ----------------
                with tc.tile_pool(name="osbp", bufs=4) as osbp, \
                     tc.tile_pool(name="outps", bufs=8, space="PSUM") as outps:
                    dma_engines = (nc.scalar, nc.sync)
                    cp_i = 0
                    for hc in range(8):
                        for qc in range(NCHUNKS):
                            for qs in range(4):
                                o_ps = outps.tile([P, 512], F32, tag="o")
                                for h in range(HPC):
                                    nc.tensor.matmul(
                                        o_ps[:],
                                        lhsT=avt_all[qc][h][:, qs * P:(qs + 1) * P],
                                        rhs=wot_sb[:, h, hc * 512:(hc + 1) * 512],
                                        start=(h == 0), stop=(h == HPC - 1))
                                o_sb = osbp.tile([P, 512], BF16, tag="osb")
                                nc.vector.tensor_copy(o_sb[:], o_ps[:])
                                dma_engines[cp_i % 2].dma_start(
                                    out_d[qc * 512 + qs * P: qc * 512 + (qs + 1) * P,
                                          hc * 512:(hc + 1) * 512],
                                    o_sb[:])
                                cp_i += 1

    nc.compile()
    return nc


def _fold(W, A, B):
    """Fold LoRA + its half/interleave permutation into the base weight."""
    BA = (B.astype(np.float64) @ A.astype(np.float64)) * LORA_SCALING
    j = np.arange(H)
    g = np.where(j < H // 2, 2 * j, 2 * (j - H // 2) + 1)
    return (W.astype(np.float64) + BA[g, :]).astype(np.float32)


def _host_consts():
    inv_freq = (1.0 / (10000.0 ** (np.arange(0, HD, 2, dtype=np.float32) / HD))).astype(np.float32)
    freqs = np.arange(S, dtype=np.float32)[:, None] * inv_freq[None, :]   # (S, 64)
    cs = np.concatenate([np.cos(freqs).T, np.sin(freqs).T], axis=0).astype(np.float32)  # (128, S)
    p = np.arange(P)[:, None]
    f = np.arange(P)[None, :]
    tri = (p > f).astype(ml_dtypes.bfloat16)              # (128, 128) kill-mask
    neye = (-2000.0 * np.eye(P)).astype(ml_dtypes.bfloat16)
    ones = np.ones((P, P), dtype=ml_dtypes.bfloat16)
    return cs, tri, neye, ones


def kernel(hidden_states, Wq, Wk, Wv, Wo, Aq, Bq, Ak, Bk, Av, Bv):
    if "nc" not in _CACHE:
        _CACHE["nc"] = _build()
    nc = _CACHE["nc"]

    x = np.ascontiguousarray(np.asarray(hidden_states, dtype=np.float32)[0])  # (S, H)
    xt_bf = np.ascontiguousarray(x.T).astype(ml_dtypes.bfloat16)

    Wq_eff = _fold(np.asarray(Wq), np.asarray(Aq), np.asarray(Bq))
    Wk_eff = _fold(np.asarray(Wk), np.asarray(Ak), np.asarray(Bk))
    Wv_eff = _fold(np.asarray(Wv), np.asarray(Av), np.asarray(Bv))
    Wo_np = np.asarray(Wo, dtype=np.float32)

    cs, tri, neye, ones = _host_consts()

    in_maps = []
    for c in range(NCORES):
        cols = slice(CW * c, CW * (c + 1))
        in_maps.append({
            "xt": xt_bf,
            "wq": np.ascontiguousarray(Wq_eff[cols].T).astype(ml_dtypes.bfloat16),
            "wk": np.ascontiguousarray(Wk_eff[cols].T).astype(ml_dtypes.bfloat16),
            "wv": np.ascontiguousarray(Wv_eff[cols].T).astype(ml_dtypes.bfloat16),
            "wot": np.ascontiguousarray(Wo_np[:, cols].T).astype(ml_dtypes.bfloat16),
            "cs": cs,
            "tri": tri,
            "neye": neye,
            "ones": ones,
        })
    _CACHE["in_maps"] = in_maps

    res = bass_utils.run_bass_kernel_spmd(nc, in_maps, core_ids=list(range(NCORES)))
    acc = np.zeros((S, H), dtype=np.float32)
    for c in range(NCORES):
        acc += res.results[c]["out"].astype(np.float32)
    return acc[None]
